# revision 40
# baseline (speedup 1.0000x reference)
"""Trainium2 Bass kernel for nn_Attention_60000693125929.

RMSNorm -> fused QKV proj -> interleaved RoPE -> causal attention -> out proj.
x: [4, 2048, 2048] f32.  8 NeuronCores: shard batch (4) x head-group (2x8 heads).

Per-core dataflow (bf16 matmul inputs, fp32 PSUM accumulation):
  1. Pass streams x tiles: ACT casts to bf16 and computes row sum-of-squares
     (Square + accum_out); PE identity-transposes into xsT quarters and
     immediately computes V = xs @ Wv, spilled to DRAM.  rinv = 1/sqrt(mean
     +eps) is folded into cos/sin tables (built once at phase end) and into
     the V normalize.
  2. qT/kT = W^T-stationary matmuls ([e, n] layout, dh on partitions);
     interleaved RoPE via a constant rotation-permutation matmul + cos/sin
     elementwise.  q/k e-tiles emitted interleaved so attention can start.
  3. Per (i-chunk, head): S^T[j,i] blocks = kT.T @ qT; causal handled by
     skipping j>i blocks and computing only the valid [lo:512] column range
     on diagonal blocks (plus a -1e30 triangle matmul for the in-block
     mask).  exp on ACT, AV + rowsum accumulate, fast reciprocal normalize.
     The PREVIOUS i-chunk's out projection (outT.T @ WoutT) is interleaved
     into the head loop at a fixed rate so the PE always has
     ACT-independent work and exp latency never stalls it; y is written out
     in [128,512] pieces on rotating DMA queues.
"""
import numpy as np
import ml_dtypes
from contextlib import ExitStack

import concourse.bass as bass
import concourse.tile as tile
from concourse import bacc, mybir
from concourse.bass_utils import run_bass_kernel_spmd

F32 = mybir.dt.float32
BF16 = mybir.dt.bfloat16
AF = mybir.ActivationFunctionType
OP = mybir.AluOpType

B, N, D, H, DH = 4, 2048, 2048, 16, 128
HPC = 8                 # heads per core
EQK = 2 * HPC * DH      # 2048 q+k columns per core
EV = HPC * DH           # 1024 v columns per core
EPS = 1.1920929e-07
SCALE = DH ** -0.5
NT = N // 128           # 16 n-tiles
DT = D // 128           # 16 d-tiles
NCH = N // 512          # 4 n-chunks

_NC_CACHE = {}


def build_nc():
    if "nc" in _NC_CACHE:
        return _NC_CACHE["nc"]
    nc = bacc.Bacc("TRN2", target_bir_lowering=False, debug=False)

    # weights arrive host-repacked so every DMA is contiguous per
    # partition (the rings are descriptor-rate-bound: 256B-run rearrange
    # loads cost ~16x more ring time than 4-8KB contiguous runs)
    x = nc.dram_tensor("x", [N, D], F32, kind="ExternalInput").ap()
    wqk = nc.dram_tensor("wqk", [128, 16, DT, 128], BF16,
                         kind="ExternalInput").ap()
    wv = nc.dram_tensor("wv", [128, 2, DT, 512], BF16,
                        kind="ExternalInput").ap()
    wout = nc.dram_tensor("wout", [128, HPC, D], BF16,
                          kind="ExternalInput").ap()
    cos_d = nc.dram_tensor("cos_t", [DH, N], BF16, kind="ExternalInput").ap()
    sin_d = nc.dram_tensor("sin_t", [DH, N], BF16, kind="ExternalInput").ap()
    pm_d = nc.dram_tensor("pm", [DH, DH], BF16, kind="ExternalInput").ap()
    id_d = nc.dram_tensor("ident", [128, 128], BF16, kind="ExternalInput").ap()
    on_d = nc.dram_tensor("onesm", [128, 128], BF16, kind="ExternalInput").ap()
    mtri_d = nc.dram_tensor("mtri", [128, 128], BF16, kind="ExternalInput").ap()
    y = nc.dram_tensor("y", [N, D], F32, kind="ExternalOutput").ap()
    # DRAM scratch for V, head-group-major ([g, nt, 128, 512] bf16) so
    # spills are contiguous and loads keep 1KB runs
    Vd = nc.dram_tensor("Vspill", [2, NT, 128, 512], BF16).ap()

    with tile.TileContext(nc) as tc, ExitStack() as ctx:
        const_p = ctx.enter_context(tc.tile_pool(name="const", bufs=1))
        small_p = ctx.enter_context(tc.tile_pool(name="small", bufs=1))
        psum = ctx.enter_context(tc.tile_pool(name="psum", bufs=4, space="PSUM"))
        po_p = ctx.enter_context(tc.tile_pool(name="pop", bufs=2, space="PSUM"))
        misc_p = ctx.enter_context(tc.tile_pool(name="miscp", bufs=2,
                                                space="PSUM"))

        id_s = const_p.tile([128, 128], BF16, tag="ident")
        on_s = const_p.tile([128, 128], BF16, tag="ones")
        mtri_s = const_p.tile([128, 128], BF16, tag="mtri")
        nc.gpsimd.dma_start(id_s[:], id_d)

        ssq = small_p.tile([128, NT], F32, tag="ssq")
        rms = small_p.tile([128, NT], F32, tag="rms")
        rinv = small_p.tile([128, NT], F32, tag="rinv")
        rinvb = small_p.tile([128, NT], BF16, tag="rinvb")
        eps_s = small_p.tile([128, 1], F32, tag="eps")
        nc.vector.memzero(eps_s[:])
        nc.vector.tensor_scalar_add(eps_s[:], eps_s[:], EPS)

        # long-lived: qkT e-tiles (q: 0..7, k: 8..15)
        qk_p = ctx.enter_context(tc.tile_pool(name="qk", bufs=1))
        qkT = [qk_p.tile([128, N], BF16, tag=f"qkT{et}", name=f"qkT{et}")
               for et in range(16)]

        # ---- phase 1: RMSNorm + transpose + V --------------------------
        with tc.tile_pool(name="xsTp", bufs=1) as xsT_p, \
             tc.tile_pool(name="ropec", bufs=1) as ropec_p:
            xsT = [xsT_p.tile([128, DT, 512], BF16, tag=f"xsT{q}",
                               name=f"xsT{q}")
                   for q in range(4)]
            cos_s = ropec_p.tile([DH, N], BF16, tag="cos")
            sin_s = ropec_p.tile([DH, N], BF16, tag="sin")
            pm_s = ropec_p.tile([DH, DH], BF16, tag="pm")
            cosr = ropec_p.tile([DH, N], BF16, tag="cosr")
            sinr = ropec_p.tile([DH, N], BF16, tag="sinr")
            with tc.tile_pool(name="ph1", bufs=2) as ph1_p, \
                 tc.tile_pool(name="ph1b", bufs=2) as ph1b_p, \
                 tc.tile_pool(name="wvp", bufs=1) as wv_p, \
                 tc.tile_pool(name="vbuf", bufs=2) as vb_p:
                wv_s = wv_p.tile([128, 2, DT, 512], BF16, tag="wv")
                # startup choreography: x0 halves lead the two fast rings
                # (paying their start latency on the critical path), wv
                # half 0 rides right behind them, half 1 on gpsimd; consts
                # are deferred out of the startup HBM window
                nc.gpsimd.dma_start(wv_s[:, 1], wv[:, 1])
                def emit_v(tv, ech):
                    qv, tqv = tv // 4, tv % 4
                    pv = psum.tile([128, 512], F32, tag="mm2", name="pv")
                    for dt_i in range(DT):
                        nc.tensor.matmul(
                            pv[:],
                            xsT[qv][:, dt_i, tqv * 128:(tqv + 1) * 128],
                            wv_s[:, ech, dt_i, :],
                            start=(dt_i == 0), stop=(dt_i == DT - 1))
                    vb = vb_p.tile([128, 512], BF16, tag="vb", name="vb")
                    nc.vector.tensor_scalar_mul(vb[:], pv[:],
                                                rinv[:, tv:tv + 1])
                    nc.gpsimd.dma_start(Vd[ech, tv], vb[:])

                for t in range(NT):
                    q, tq = t // 4, t % 4
                    xt = ph1_p.tile([128, D], F32, tag="xin")
                    if t == 0:
                        # split first tile across two rings; alternate the
                        # casts over ACT/DVE so the PE starts sooner
                        nc.sync.dma_start(xt[:, :1024], x[0:128, :1024])
                        nc.scalar.dma_start(xt[:, 1024:], x[0:128, 1024:])
                        nc.sync.dma_start(wv_s[:, 0, 0:8], wv[:, 0, 0:8])
                        nc.scalar.dma_start(wv_s[:, 0, 8:16],
                                            wv[:, 0, 8:16])
                    else:
                        nc.sync.dma_start(xt[:],
                                          x[t * 128:(t + 1) * 128, :])
                    if t == 2:
                        # consts only needed from phase-1 end onward: keep
                        # them out of the startup HBM window
                        nc.gpsimd.dma_start(cos_s[:], cos_d)
                        nc.gpsimd.dma_start(sin_s[:], sin_d)
                    if t == 3:
                        nc.gpsimd.dma_start(pm_s[:], pm_d)
                        nc.gpsimd.dma_start(on_s[:], on_d)
                        nc.gpsimd.dma_start(mtri_s[:], mtri_d)
                    xr = ph1b_p.tile([128, D], BF16, tag="xraw")
                    for c in range(4):
                        if c % 2 == 1:
                            nc.vector.tensor_copy(
                                xr[:, c * 512:(c + 1) * 512],
                                xt[:, c * 512:(c + 1) * 512])
                        else:
                            nc.scalar.activation(
                                xr[:, c * 512:(c + 1) * 512],
                                xt[:, c * 512:(c + 1) * 512], AF.Copy)
                    # row sum-of-squares on ACT (accumulate over free dim);
                    # in-place on xt -- the casts above already consumed it
                    # and ACT executes in order
                    nc.scalar.activation(xt[:], xt[:], AF.Square,
                                         accum_out=ssq[:, t:t + 1])
                    nc.scalar.activation(rms[:, t:t + 1], ssq[:, t:t + 1],
                                         AF.Sqrt, bias=eps_s[:],
                                         scale=1.0 / D)
                    nc.vector.reciprocal(rinv[:, t:t + 1], rms[:, t:t + 1])
                    for q4 in range(4):
                        pt = psum.tile([128, 4, 128], BF16, tag="mm2")
                        for j in range(4):
                            dt_i = 4 * q4 + j
                            nc.tensor.transpose(
                                pt[:, j, :],
                                xr[:, dt_i * 128:(dt_i + 1) * 128], id_s[:])
                        nc.vector.tensor_copy(
                            xsT[q][:, 4 * q4:4 * q4 + 4,
                                   tq * 128:(tq + 1) * 128],
                            pt[:])
                    # V lags the transposes (ech0 by 1 tile, ech1 by 2) so
                    # early PE work only needs the first wv half while the
                    # rest of the weights stream in
                    if t > 0:
                        emit_v(t - 1, 0)
                    if t > 1:
                        emit_v(t - 2, 1)
                emit_v(NT - 1, 0)
                emit_v(NT - 2, 1)
                emit_v(NT - 1, 1)
                # rinv -> row form -> cos/sin tables pre-scaled by rinv[n]
                nc.vector.tensor_copy(rinvb[:], rinv[:])
                for c in range(NCH):
                    prow = misc_p.tile([1, 512], F32, tag="av")
                    for tq in range(4):
                        t = 4 * c + tq
                        nc.tensor.matmul(
                            prow[:, tq * 128:(tq + 1) * 128],
                            rinvb[:, t:t + 1], id_s[:],
                            start=True, stop=True)
                    rrow = vb_p.tile([1, 512], BF16, tag="rrow")
                    nc.vector.tensor_copy(rrow[:], prow[:])
                    pb = misc_p.tile([128, 512], F32, tag="av")
                    nc.tensor.matmul(pb[:], on_s[0:1, :], rrow[:],
                                     start=True, stop=True)
                    sl = slice(c * 512, (c + 1) * 512)
                    nc.vector.tensor_tensor(cosr[:, sl], cos_s[:, sl],
                                            pb[:], OP.mult)
                    nc.vector.tensor_tensor(sinr[:, sl], sin_s[:, sl],
                                            pb[:], OP.mult)

            # ---- phase 2: qkT + RoPE (inside xsT scope) -----------------
            with tc.tile_pool(name="wqkp", bufs=2) as wqk_p, \
                 tc.tile_pool(name="rope", bufs=4) as rope_p:
                order = [v for pair in zip(range(8), range(8, 16))
                         for v in pair]
                def emit_rope(st):
                    et_, n0_, raw_ = st
                    prot = psum.tile([128, 512], F32, tag="mm2")
                    nc.tensor.matmul(prot[:], pm_s[:], raw_[:],
                                     start=True, stop=True)
                    t1 = rope_p.tile([128, 512], BF16, tag="t1")
                    nc.vector.tensor_tensor(
                        t1[:], raw_[:], cosr[:, n0_:n0_ + 512], OP.mult)
                    t2 = rope_p.tile([128, 512], BF16, tag="t2")
                    nc.vector.tensor_tensor(
                        t2[:], prot[:], sinr[:, n0_:n0_ + 512], OP.mult)
                    nc.vector.tensor_add(
                        qkT[et_][:, n0_:n0_ + 512], t1[:], t2[:])

                pending = None
                for et in order:
                    wt = wqk_p.tile([128, DT, 128], BF16, tag="wqk")
                    nc.sync.dma_start(wt[:], wqk[:, et])
                    for nch in range(NCH):
                        n0 = nch * 512
                        pq = psum.tile([128, 512], F32, tag="mm2")
                        for dt_i in range(DT):
                            nc.tensor.matmul(
                                pq[:], wt[:, dt_i, :],
                                xsT[nch][:, dt_i, :],
                                start=(dt_i == 0), stop=(dt_i == DT - 1))
                        raw = rope_p.tile([128, 512], BF16, tag="raw")
                        nc.scalar.activation(raw[:], pq[:], AF.Copy,
                                             bias=0.0, scale=1.0)
                        # lag the rot matmul one step so the PE never waits
                        # on the ACT copy in its in-order queue
                        if pending is not None:
                            emit_rope(pending)
                        pending = (et, n0, raw)
                if pending is not None:
                    emit_rope(pending)

        # ---- phase 3+4: causal attention + out projection ---------------
        with tc.tile_pool(name="outp", bufs=1) as out_p, \
             tc.tile_pool(name="exps", bufs=8) as exps_p, \
             tc.tile_pool(name="att", bufs=3) as att_p, \
             tc.tile_pool(name="vstr", bufs=3) as vs_p, \
             tc.tile_pool(name="woutp", bufs=1) as wo_p, \
             tc.tile_pool(name="ybufp", bufs=3) as y_p:
            wo_s = wo_p.tile([128, HPC, D], BF16, tag="wo")
            nc.gpsimd.dma_start(wo_s[:], wout)
            outT = [out_p.tile([128, HPC, 512], BF16, tag=f"outT{q}",
                                name=f"outT{q}")
                    for q in range(4)]
            deferred = [None]

            def finalize_head(st):
                ic_, h_, po_, racc_ = st
                pr = misc_p.tile([128, 512], F32, tag="av")
                nc.tensor.matmul(pr[:], on_s[:], racc_[:],
                                 start=True, stop=True)
                rec = att_p.tile([128, 512], F32, tag="rec")
                rsc = att_p.tile([128, 512], F32, tag="rsc")
                nc.vector.reciprocal_approx_accurate(rec[:], pr[:], rsc[:])
                nc.vector.tensor_tensor(
                    outT[ic_][:, h_, :], po_[:], rec[:], OP.mult)

            # interleaved out-projection of the previous i-chunk
            ops = dict(pieces=[], carry=0.0, rate=0.0, py=None, et=0, dq=0)
            dmaq = [nc.sync, nc.gpsimd]

            def op_begin(pic, steps):
                ops["pieces"] = [(4 * pic + tq, dch)
                                 for tq in range(4) for dch in range(4)]
                ops["rate"] = (16.0 * HPC) / steps
                ops["carry"] = 0.0
                ops["py"] = None
                ops["pic"] = pic

            def op_step(force=False):
                if force:
                    n = 1 << 30
                else:
                    ops["carry"] += ops["rate"]
                    n = int(ops["carry"])
                    ops["carry"] -= n
                while n > 0 and ops["pieces"]:
                    t, dch = ops["pieces"][0]
                    if ops["py"] is None:
                        ops["py"] = misc_p.tile([128, 512], F32, tag="av",
                                                name="oppy")
                        ops["et"] = 0
                    et = ops["et"]
                    tq = t % 4
                    nc.tensor.matmul(
                        ops["py"][:],
                        outT[ops["pic"]][:, et, tq * 128:(tq + 1) * 128],
                        wo_s[:, et, dch * 512:(dch + 1) * 512],
                        start=(et == 0), stop=(et == HPC - 1))
                    ops["et"] += 1
                    n -= 1
                    if ops["et"] == HPC:
                        yb = y_p.tile([128, 512], F32, tag="yb")
                        nc.vector.tensor_copy(yb[:], ops["py"][:])
                        qd = dmaq[ops["dq"] % 2]
                        ops["dq"] += 1
                        qd.dma_start(
                            y[t * 128:(t + 1) * 128,
                              dch * 512:(dch + 1) * 512], yb[:])
                        ops["pieces"].pop(0)
                        ops["py"] = None

            # vstrip loads cover 4 heads at once (1KB descriptor runs,
            # 1/4 the descriptor count per head) and split the jt range
            # across the sync and gpsimd rings.  bufs=2 double-buffers the
            # two 4-head groups of an i-chunk.
            def emit_vload(ic_, g_):
                njt_ = 4 * ic_ + 4
                vt = vs_p.tile([128, NT, 512], BF16, tag="vstr",
                               name="vload")
                half = njt_ // 2
                nc.sync.dma_start(
                    vt[:, :half, :],
                    Vd[g_, :half].rearrange("jt p e -> p jt e"))
                nc.gpsimd.dma_start(
                    vt[:, half:njt_, :],
                    Vd[g_, half:njt_].rearrange("jt p e -> p jt e"))
                return vt

            vload = [emit_vload(0, 0), None]
            for ic in range(NCH):
                i0 = ic * 512
                njt = 4 * ic + 4
                if ic >= 1:
                    op_begin(ic - 1, 8 * (njt + 4))
                for h in range(HPC):
                    if h == 0:
                        vload[1] = emit_vload(ic, 1)
                    if h == 2 and ic + 1 < NCH:
                        vload[0] = emit_vload(ic + 1, 0)
                    vstrip = vload[h // 4]
                    hc = (h % 4) * 128
                    po = po_p.tile([128, 512], F32, tag="av")
                    # QK+exp run 3 tiles ahead of AV so the PE's in-order
                    # queue never waits on the ACT exp.  Row sums accumulate
                    # on DVE (racc) -> a single ones-matmul per (ic, h).
                    racc = att_p.tile([128, 512], BF16, tag="racc")
                    pend = []

                    def drain_one():
                        jt_, lo_, es_ = pend.pop(0)
                        nc.tensor.matmul(
                            po[:, lo_:512],
                            vstrip[:, jt_, hc:hc + 128],
                            es_[:, lo_:512],
                            start=(jt_ == 0), stop=(jt_ == njt - 1))

                    for jt in range(njt):
                        r = jt - 4 * ic
                        lo = max(0, 128 * r)
                        psq = psum.tile([128, 512], F32, tag="mm2")
                        nc.tensor.matmul(
                            psq[:, lo:512],
                            qkT[HPC + h][:, jt * 128:(jt + 1) * 128],
                            qkT[h][:, i0 + lo:i0 + 512],
                            start=True, stop=(r < 0))
                        if r >= 0:
                            nc.tensor.matmul(
                                psq[:, lo:lo + 128],
                                mtri_s[:], id_s[:], start=False, stop=True)
                        es = exps_p.tile([128, 512], BF16, tag="es")
                        nc.scalar.activation(es[:, lo:], psq[:, lo:512],
                                             AF.Exp, bias=0.0, scale=SCALE)
                        if jt == 0:
                            nc.vector.tensor_copy(racc[:], es[:])
                        else:
                            nc.vector.tensor_add(racc[:, lo:], racc[:, lo:],
                                                 es[:, lo:])
                        pend.append((jt, lo, es))
                        if len(pend) > 3:
                            drain_one()
                        op_step()
                        # previous head finalizes mid-stream so its rowsum
                        # matmul never stalls the PE on the DVE racc chain
                        if jt == min(2, njt - 1) and deferred[0] is not None:
                            finalize_head(deferred[0])
                            deferred[0] = None
                    while pend:
                        drain_one()
                        op_step()
                    deferred[0] = (ic, h, po, racc)
                if deferred[0] is not None:
                    finalize_head(deferred[0])
                    deferred[0] = None
                op_step(force=True)
            # tail: out projection of the last i-chunk
            op_begin(NCH - 1, 1)
            op_step(force=True)

    nc.compile()
    _NC_CACHE["nc"] = nc
    return nc


def _host_prep(rotary_pos_emb, w_rms, w_qkv, w_out):
    bf = ml_dtypes.bfloat16
    cos_t = np.ascontiguousarray(np.cos(rotary_pos_emb).T).astype(bf)
    sin_t = np.ascontiguousarray(np.sin(rotary_pos_emb).T).astype(bf)
    # rotate_half as a matrix: rot(t)[2i] = -t[2i+1], rot(t)[2i+1] = t[2i]
    P = np.zeros((DH, DH), np.float32)
    for i in range(DH // 2):
        P[2 * i, 2 * i + 1] = -1.0
        P[2 * i + 1, 2 * i] = 1.0
    pm = np.ascontiguousarray(P.T).astype(bf)       # lhsT for rot matmul
    ident = np.eye(128, dtype=bf)
    onesm = np.ones((128, 128), dtype=bf)
    # mtri = M.T with M[jj, cc] = -1e30 where cc < jj (strict lower tri)
    M = np.where(np.arange(128)[None, :] < np.arange(128)[:, None],
                 np.float32(-1e30), np.float32(0.0))
    mtri = np.ascontiguousarray(M.T).astype(bf)

    Ws = (w_qkv * w_rms[None, :]).astype(np.float32)  # fold RMSNorm weight
    per_core = []
    for g in range(2):
        rq = Ws[g * 1024:(g + 1) * 1024]              # q rows, heads 8g..
        rk = Ws[D + g * 1024:D + (g + 1) * 1024]      # k rows
        rv = Ws[2 * D + g * 1024:2 * D + (g + 1) * 1024]
        wqk_g = np.concatenate([rq, rk], 0).T.astype(bf)   # [D, 2048]
        wv_g = rv.T.astype(bf)                             # [D, 1024]
        wout_g = w_out[:, g * 1024:(g + 1) * 1024].T.astype(bf)  # [EV, D]
        # repack so device DMAs are contiguous per partition:
        # wqk [D=(dt p), e] -> [p, et, dt, 128]
        wqk_pk = np.ascontiguousarray(
            wqk_g.reshape(16, 128, 16, 128).transpose(1, 2, 0, 3))
        # wv [D=(dt p), e] -> [p, ech, dt, 512]
        wv_pk = np.ascontiguousarray(
            wv_g.reshape(16, 128, 2, 512).transpose(1, 2, 0, 3))
        # wout [EV=(et p), d] -> [p, et, d]
        wout_pk = np.ascontiguousarray(
            wout_g.reshape(8, 128, 2048).transpose(1, 0, 2))
        per_core.append(dict(wqk=wqk_pk, wv=wv_pk, wout=wout_pk,
                             cos_t=cos_t, sin_t=sin_t, pm=pm, ident=ident,
                             onesm=onesm, mtri=mtri))
    return per_core


def kernel(x, rotary_pos_emb, w_rms, w_qkv, w_out, _run=None):
    x = np.asarray(x, np.float32)
    rotary_pos_emb = np.asarray(rotary_pos_emb, np.float32)
    w_rms = np.asarray(w_rms, np.float32)
    w_qkv = np.asarray(w_qkv, np.float32)
    w_out = np.asarray(w_out, np.float32)

    nc = build_nc()
    groups = _host_prep(rotary_pos_emb, w_rms, w_qkv, w_out)
    in_maps = []
    for b in range(B):
        for g in range(2):
            m = dict(groups[g])
            m["x"] = np.ascontiguousarray(x[b])
            in_maps.append(m)
    if _run is None:
        res = run_bass_kernel_spmd(nc, in_maps, core_ids=list(range(8)))
        results = res.results
    else:
        results = _run(nc, in_maps)

    y = np.empty((B, N, D), np.float32)
    for b in range(B):
        y[b] = results[2 * b]["y"] + results[2 * b + 1]["y"]
    return y


# revision 43
# speedup vs baseline: 1.1852x; 1.1852x over previous
"""Trainium2 Bass kernel for nn_Attention_60000693125929.

RMSNorm -> fused QKV proj -> interleaved RoPE -> causal attention -> out proj.
x: [4, 2048, 2048] f32.  8 NeuronCores: shard batch (4) x head-group (2x8 heads).

Per-core dataflow (bf16 matmul inputs, fp32 PSUM accumulation):
  1. Pass streams x tiles: ACT casts to bf16 and computes row sum-of-squares
     (Square + accum_out); PE identity-transposes into xsT quarters and
     immediately computes V = xs @ Wv, spilled to DRAM.  rinv = 1/sqrt(mean
     +eps) is folded into cos/sin tables (built once at phase end) and into
     the V normalize.
  2. qT/kT = W^T-stationary matmuls ([e, n] layout, dh on partitions);
     interleaved RoPE via a constant rotation-permutation matmul + cos/sin
     elementwise.  q/k e-tiles emitted interleaved so attention can start.
  3. Per (i-chunk, head): S^T[j,i] blocks = kT.T @ qT; causal handled by
     skipping j>i blocks and computing only the valid [lo:512] column range
     on diagonal blocks (plus a -1e30 triangle matmul for the in-block
     mask).  exp on ACT, AV + rowsum accumulate, fast reciprocal normalize.
     The PREVIOUS i-chunk's out projection (outT.T @ WoutT) is interleaved
     into the head loop at a fixed rate so the PE always has
     ACT-independent work and exp latency never stalls it; y is written out
     in [128,512] pieces on rotating DMA queues.
"""
import numpy as np
import ml_dtypes
from contextlib import ExitStack

import concourse.bass as bass
import concourse.tile as tile
from concourse import bacc, mybir
from concourse.bass_utils import run_bass_kernel_spmd

F32 = mybir.dt.float32
BF16 = mybir.dt.bfloat16
AF = mybir.ActivationFunctionType
OP = mybir.AluOpType

B, N, D, H, DH = 4, 2048, 2048, 16, 128
HPC = 8                 # heads per core
EQK = 2 * HPC * DH      # 2048 q+k columns per core
EV = HPC * DH           # 1024 v columns per core
EPS = 1.1920929e-07
SCALE = DH ** -0.5
NT = N // 128           # 16 n-tiles
DT = D // 128           # 16 d-tiles
NCH = N // 512          # 4 n-chunks

_NC_CACHE = {}


def build_nc():
    if "nc" in _NC_CACHE:
        return _NC_CACHE["nc"]
    nc = bacc.Bacc("TRN2", target_bir_lowering=False, debug=False)

    # weights arrive host-repacked so every DMA is contiguous per
    # partition (the rings are descriptor-rate-bound: 256B-run rearrange
    # loads cost ~16x more ring time than 4-8KB contiguous runs)
    x = nc.dram_tensor("x", [N, D], F32, kind="ExternalInput").ap()
    wqk = nc.dram_tensor("wqk", [128, 16, DT, 128], BF16,
                         kind="ExternalInput").ap()
    wv = nc.dram_tensor("wv", [128, 2, DT, 512], BF16,
                        kind="ExternalInput").ap()
    wout = nc.dram_tensor("wout", [128, HPC, D], BF16,
                          kind="ExternalInput").ap()
    cos_d = nc.dram_tensor("cos_t", [DH, N], BF16, kind="ExternalInput").ap()
    sin_d = nc.dram_tensor("sin_t", [DH, N], BF16, kind="ExternalInput").ap()
    pm_d = nc.dram_tensor("pm", [DH, DH], BF16, kind="ExternalInput").ap()
    id_d = nc.dram_tensor("ident", [128, 128], BF16, kind="ExternalInput").ap()
    on_d = nc.dram_tensor("onesm", [128, 128], BF16, kind="ExternalInput").ap()
    mtri_d = nc.dram_tensor("mtri", [128, 128], BF16, kind="ExternalInput").ap()
    y = nc.dram_tensor("y", [N, D], F32, kind="ExternalOutput").ap()
    # DRAM scratch for V, head-group-major ([g, nt, 128, 512] bf16) so
    # spills are contiguous and loads keep 1KB runs
    Vd = nc.dram_tensor("Vspill", [2, NT, 128, 512], BF16).ap()

    with tile.TileContext(nc) as tc, ExitStack() as ctx:
        const_p = ctx.enter_context(tc.tile_pool(name="const", bufs=1))
        small_p = ctx.enter_context(tc.tile_pool(name="small", bufs=1))
        psum = ctx.enter_context(tc.tile_pool(name="psum", bufs=4, space="PSUM"))
        po_p = ctx.enter_context(tc.tile_pool(name="pop", bufs=2, space="PSUM"))
        misc_p = ctx.enter_context(tc.tile_pool(name="miscp", bufs=2,
                                                space="PSUM"))

        id_s = const_p.tile([128, 128], BF16, tag="ident")
        on_s = const_p.tile([128, 128], BF16, tag="ones")
        mtri_s = const_p.tile([128, 128], BF16, tag="mtri")
        nc.gpsimd.dma_start(id_s[:], id_d)

        ssq = small_p.tile([128, NT], F32, tag="ssq")
        rms = small_p.tile([128, NT], F32, tag="rms")
        rinv = small_p.tile([128, NT], F32, tag="rinv")
        rinvb = small_p.tile([128, NT], BF16, tag="rinvb")
        eps_s = small_p.tile([128, 1], F32, tag="eps")
        nc.vector.memzero(eps_s[:])
        nc.vector.tensor_scalar_add(eps_s[:], eps_s[:], EPS)

        # long-lived: qkT e-tiles (q: 0..7, k: 8..15)
        qk_p = ctx.enter_context(tc.tile_pool(name="qk", bufs=1))
        qkT = [qk_p.tile([128, N], BF16, tag=f"qkT{et}", name=f"qkT{et}")
               for et in range(16)]

        # ---- phase 1: RMSNorm + transpose + V --------------------------
        with tc.tile_pool(name="xsTp", bufs=1) as xsT_p, \
             tc.tile_pool(name="ropec", bufs=1) as ropec_p:
            xsT = [xsT_p.tile([128, DT, 512], BF16, tag=f"xsT{q}",
                               name=f"xsT{q}")
                   for q in range(4)]
            cos_s = ropec_p.tile([DH, N], BF16, tag="cos")
            sin_s = ropec_p.tile([DH, N], BF16, tag="sin")
            pm_s = ropec_p.tile([DH, DH], BF16, tag="pm")
            cosr = ropec_p.tile([DH, N], BF16, tag="cosr")
            sinr = ropec_p.tile([DH, N], BF16, tag="sinr")
            with tc.tile_pool(name="ph1", bufs=2) as ph1_p, \
                 tc.tile_pool(name="ph1b", bufs=2) as ph1b_p, \
                 tc.tile_pool(name="wvp", bufs=1) as wv_p, \
                 tc.tile_pool(name="vbuf", bufs=2) as vb_p:
                # wv halves as SEPARATE tiles: DMA-written tiles have
                # whole-tile dependency granularity, so ech0's matmuls must
                # not share a tile with the late-arriving ech1 half.
                # Startup choreography: x0 halves lead the two fast rings,
                # wv half 0 rides right behind them, half 1 on gpsimd;
                # consts are deferred out of the startup HBM window.
                wv_s0 = wv_p.tile([128, DT, 512], BF16, tag="wv0")
                wv_s1 = wv_p.tile([128, DT, 512], BF16, tag="wv1")
                wv_ss = [wv_s0, wv_s1]
                nc.gpsimd.dma_start(wv_s1[:], wv[:, 1])
                def emit_v(tv, ech):
                    qv, tqv = tv // 4, tv % 4
                    pv = psum.tile([128, 512], F32, tag="mm2", name="pv")
                    for dt_i in range(DT):
                        nc.tensor.matmul(
                            pv[:],
                            xsT[qv][:, dt_i, tqv * 128:(tqv + 1) * 128],
                            wv_ss[ech][:, dt_i, :],
                            start=(dt_i == 0), stop=(dt_i == DT - 1))
                    vb = vb_p.tile([128, 512], BF16, tag="vb", name="vb")
                    nc.vector.tensor_scalar_mul(vb[:], pv[:],
                                                rinv[:, tv:tv + 1])
                    nc.gpsimd.dma_start(Vd[ech, tv], vb[:])

                for t in range(NT):
                    q, tq = t // 4, t % 4
                    xt = ph1_p.tile([128, D], F32, tag="xin")
                    if t == 0:
                        # split first tile across two rings; alternate the
                        # casts over ACT/DVE so the PE starts sooner
                        nc.sync.dma_start(xt[:, :1024], x[0:128, :1024])
                        nc.scalar.dma_start(xt[:, 1024:], x[0:128, 1024:])
                        nc.sync.dma_start(wv_s0[:, 0:8], wv[:, 0, 0:8])
                        nc.scalar.dma_start(wv_s0[:, 8:16],
                                            wv[:, 0, 8:16])
                    else:
                        nc.sync.dma_start(xt[:],
                                          x[t * 128:(t + 1) * 128, :])
                    if t == 2:
                        # consts only needed from phase-1 end onward: keep
                        # them out of the startup HBM window
                        nc.gpsimd.dma_start(cos_s[:], cos_d)
                        nc.gpsimd.dma_start(sin_s[:], sin_d)
                    if t == 3:
                        nc.gpsimd.dma_start(pm_s[:], pm_d)
                        nc.gpsimd.dma_start(on_s[:], on_d)
                        nc.gpsimd.dma_start(mtri_s[:], mtri_d)
                    xr = ph1b_p.tile([128, D], BF16, tag="xraw")
                    for c in range(4):
                        if c % 2 == 1:
                            nc.vector.tensor_copy(
                                xr[:, c * 512:(c + 1) * 512],
                                xt[:, c * 512:(c + 1) * 512])
                        else:
                            nc.scalar.activation(
                                xr[:, c * 512:(c + 1) * 512],
                                xt[:, c * 512:(c + 1) * 512], AF.Copy)
                    # row sum-of-squares on ACT (accumulate over free dim);
                    # in-place on xt -- the casts above already consumed it
                    # and ACT executes in order
                    nc.scalar.activation(xt[:], xt[:], AF.Square,
                                         accum_out=ssq[:, t:t + 1])
                    nc.scalar.activation(rms[:, t:t + 1], ssq[:, t:t + 1],
                                         AF.Sqrt, bias=eps_s[:],
                                         scale=1.0 / D)
                    nc.vector.reciprocal(rinv[:, t:t + 1], rms[:, t:t + 1])
                    for q4 in range(4):
                        pt = psum.tile([128, 4, 128], BF16, tag="mm2")
                        for j in range(4):
                            dt_i = 4 * q4 + j
                            nc.tensor.transpose(
                                pt[:, j, :],
                                xr[:, dt_i * 128:(dt_i + 1) * 128], id_s[:])
                        nc.vector.tensor_copy(
                            xsT[q][:, 4 * q4:4 * q4 + 4,
                                   tq * 128:(tq + 1) * 128],
                            pt[:])
                    # V lags the transposes (ech0 by 1 tile, ech1 by 2) so
                    # early PE work only needs the first wv half while the
                    # rest of the weights stream in
                    if t > 0:
                        emit_v(t - 1, 0)
                    if t > 1:
                        emit_v(t - 2, 1)
                emit_v(NT - 1, 0)
                emit_v(NT - 2, 1)
                emit_v(NT - 1, 1)
                # rinv -> row form -> cos/sin tables pre-scaled by rinv[n]
                nc.vector.tensor_copy(rinvb[:], rinv[:])
                for c in range(NCH):
                    prow = misc_p.tile([1, 512], F32, tag="av")
                    for tq in range(4):
                        t = 4 * c + tq
                        nc.tensor.matmul(
                            prow[:, tq * 128:(tq + 1) * 128],
                            rinvb[:, t:t + 1], id_s[:],
                            start=True, stop=True)
                    rrow = vb_p.tile([1, 512], BF16, tag="rrow")
                    nc.vector.tensor_copy(rrow[:], prow[:])
                    pb = misc_p.tile([128, 512], F32, tag="av")
                    nc.tensor.matmul(pb[:], on_s[0:1, :], rrow[:],
                                     start=True, stop=True)
                    sl = slice(c * 512, (c + 1) * 512)
                    nc.vector.tensor_tensor(cosr[:, sl], cos_s[:, sl],
                                            pb[:], OP.mult)
                    nc.vector.tensor_tensor(sinr[:, sl], sin_s[:, sl],
                                            pb[:], OP.mult)

            # ---- phase 2: qkT + RoPE (inside xsT scope) -----------------
            with tc.tile_pool(name="wqkp", bufs=2) as wqk_p, \
                 tc.tile_pool(name="rope", bufs=4) as rope_p:
                order = [v for pair in zip(range(8), range(8, 16))
                         for v in pair]
                def emit_rope(st):
                    et_, n0_, raw_ = st
                    prot = psum.tile([128, 512], F32, tag="mm2")
                    nc.tensor.matmul(prot[:], pm_s[:], raw_[:],
                                     start=True, stop=True)
                    t1 = rope_p.tile([128, 512], BF16, tag="t1")
                    nc.vector.tensor_tensor(
                        t1[:], raw_[:], cosr[:, n0_:n0_ + 512], OP.mult)
                    t2 = rope_p.tile([128, 512], BF16, tag="t2")
                    nc.vector.tensor_tensor(
                        t2[:], prot[:], sinr[:, n0_:n0_ + 512], OP.mult)
                    nc.vector.tensor_add(
                        qkT[et_][:, n0_:n0_ + 512], t1[:], t2[:])

                pending = None
                for et in order:
                    wt = wqk_p.tile([128, DT, 128], BF16, tag="wqk")
                    nc.sync.dma_start(wt[:], wqk[:, et])
                    for nch in range(NCH):
                        n0 = nch * 512
                        pq = psum.tile([128, 512], F32, tag="mm2")
                        for dt_i in range(DT):
                            nc.tensor.matmul(
                                pq[:], wt[:, dt_i, :],
                                xsT[nch][:, dt_i, :],
                                start=(dt_i == 0), stop=(dt_i == DT - 1))
                        raw = rope_p.tile([128, 512], BF16, tag="raw")
                        nc.scalar.activation(raw[:], pq[:], AF.Copy,
                                             bias=0.0, scale=1.0)
                        # lag the rot matmul one step so the PE never waits
                        # on the ACT copy in its in-order queue
                        if pending is not None:
                            emit_rope(pending)
                        pending = (et, n0, raw)
                if pending is not None:
                    emit_rope(pending)

        # ---- phase 3+4: causal attention + out projection ---------------
        with tc.tile_pool(name="outp", bufs=1) as out_p, \
             tc.tile_pool(name="exps", bufs=8) as exps_p, \
             tc.tile_pool(name="att", bufs=3) as att_p, \
             tc.tile_pool(name="vstr", bufs=3) as vs_p, \
             tc.tile_pool(name="woutp", bufs=1) as wo_p, \
             tc.tile_pool(name="ybufp", bufs=3) as y_p:
            wo_s = wo_p.tile([128, HPC, D], BF16, tag="wo")
            nc.gpsimd.dma_start(wo_s[:], wout)
            outT = [out_p.tile([128, HPC, 512], BF16, tag=f"outT{q}",
                                name=f"outT{q}")
                    for q in range(4)]
            deferred = [None]

            def finalize_head(st):
                ic_, h_, po_, racc_ = st
                pr = misc_p.tile([128, 512], F32, tag="av")
                nc.tensor.matmul(pr[:], on_s[:], racc_[:],
                                 start=True, stop=True)
                rec = att_p.tile([128, 512], F32, tag="rec")
                rsc = att_p.tile([128, 512], F32, tag="rsc")
                nc.vector.reciprocal_approx_accurate(rec[:], pr[:], rsc[:])
                nc.vector.tensor_tensor(
                    outT[ic_][:, h_, :], po_[:], rec[:], OP.mult)

            # interleaved out-projection of the previous i-chunk
            ops = dict(pieces=[], carry=0.0, rate=0.0, py=None, et=0, dq=0)
            dmaq = [nc.sync, nc.gpsimd]

            def op_begin(pic, steps):
                ops["pieces"] = [(4 * pic + tq, dch)
                                 for tq in range(4) for dch in range(4)]
                ops["rate"] = (16.0 * HPC) / steps
                ops["carry"] = 0.0
                ops["py"] = None
                ops["pic"] = pic

            def op_step(force=False):
                if force:
                    n = 1 << 30
                else:
                    ops["carry"] += ops["rate"]
                    n = int(ops["carry"])
                    ops["carry"] -= n
                while n > 0 and ops["pieces"]:
                    t, dch = ops["pieces"][0]
                    if ops["py"] is None:
                        ops["py"] = misc_p.tile([128, 512], F32, tag="av",
                                                name="oppy")
                        ops["et"] = 0
                    et = ops["et"]
                    tq = t % 4
                    nc.tensor.matmul(
                        ops["py"][:],
                        outT[ops["pic"]][:, et, tq * 128:(tq + 1) * 128],
                        wo_s[:, et, dch * 512:(dch + 1) * 512],
                        start=(et == 0), stop=(et == HPC - 1))
                    ops["et"] += 1
                    n -= 1
                    if ops["et"] == HPC:
                        yb = y_p.tile([128, 512], F32, tag="yb")
                        nc.vector.tensor_copy(yb[:], ops["py"][:])
                        qd = dmaq[ops["dq"] % 2]
                        ops["dq"] += 1
                        qd.dma_start(
                            y[t * 128:(t + 1) * 128,
                              dch * 512:(dch + 1) * 512], yb[:])
                        ops["pieces"].pop(0)
                        ops["py"] = None

            # vstrip loads cover 4 heads at once (1KB descriptor runs,
            # 1/4 the descriptor count per head) and split the jt range
            # across the sync and gpsimd rings.  bufs=2 double-buffers the
            # two 4-head groups of an i-chunk.
            def emit_vload(ic_, g_):
                njt_ = 4 * ic_ + 4
                vt = vs_p.tile([128, NT, 512], BF16, tag="vstr",
                               name="vload")
                half = njt_ // 2
                nc.sync.dma_start(
                    vt[:, :half, :],
                    Vd[g_, :half].rearrange("jt p e -> p jt e"))
                nc.gpsimd.dma_start(
                    vt[:, half:njt_, :],
                    Vd[g_, half:njt_].rearrange("jt p e -> p jt e"))
                return vt

            vload = [emit_vload(0, 0), None]
            for ic in range(NCH):
                i0 = ic * 512
                njt = 4 * ic + 4
                if ic >= 1:
                    op_begin(ic - 1, 8 * (njt + 4))
                for h in range(HPC):
                    if h == 0:
                        vload[1] = emit_vload(ic, 1)
                    if h == 2 and ic + 1 < NCH:
                        vload[0] = emit_vload(ic + 1, 0)
                    vstrip = vload[h // 4]
                    hc = (h % 4) * 128
                    po = po_p.tile([128, 512], F32, tag="av")
                    # QK+exp run 3 tiles ahead of AV so the PE's in-order
                    # queue never waits on the ACT exp.  Row sums accumulate
                    # on DVE (racc) -> a single ones-matmul per (ic, h).
                    racc = att_p.tile([128, 512], BF16, tag="racc")
                    pend = []

                    def drain_one():
                        jt_, lo_, es_ = pend.pop(0)
                        nc.tensor.matmul(
                            po[:, lo_:512],
                            vstrip[:, jt_, hc:hc + 128],
                            es_[:, lo_:512],
                            start=(jt_ == 0), stop=(jt_ == njt - 1))

                    for jt in range(njt):
                        r = jt - 4 * ic
                        lo = max(0, 128 * r)
                        psq = psum.tile([128, 512], F32, tag="mm2")
                        nc.tensor.matmul(
                            psq[:, lo:512],
                            qkT[HPC + h][:, jt * 128:(jt + 1) * 128],
                            qkT[h][:, i0 + lo:i0 + 512],
                            start=True, stop=(r < 0))
                        if r >= 0:
                            nc.tensor.matmul(
                                psq[:, lo:lo + 128],
                                mtri_s[:], id_s[:], start=False, stop=True)
                        es = exps_p.tile([128, 512], BF16, tag="es")
                        nc.scalar.activation(es[:, lo:], psq[:, lo:512],
                                             AF.Exp, bias=0.0, scale=SCALE)
                        if jt == 0:
                            nc.vector.tensor_copy(racc[:], es[:])
                        else:
                            nc.vector.tensor_add(racc[:, lo:], racc[:, lo:],
                                                 es[:, lo:])
                        pend.append((jt, lo, es))
                        if len(pend) > 3:
                            drain_one()
                        op_step()
                        # previous head finalizes mid-stream so its rowsum
                        # matmul never stalls the PE on the DVE racc chain
                        if jt == min(2, njt - 1) and deferred[0] is not None:
                            finalize_head(deferred[0])
                            deferred[0] = None
                    while pend:
                        drain_one()
                        op_step()
                    deferred[0] = (ic, h, po, racc)
                if deferred[0] is not None:
                    finalize_head(deferred[0])
                    deferred[0] = None
                op_step(force=True)
            # tail: out projection of the last i-chunk
            op_begin(NCH - 1, 1)
            op_step(force=True)

    nc.compile()
    _NC_CACHE["nc"] = nc
    return nc


def _host_prep(rotary_pos_emb, w_rms, w_qkv, w_out):
    bf = ml_dtypes.bfloat16
    cos_t = np.ascontiguousarray(np.cos(rotary_pos_emb).T).astype(bf)
    sin_t = np.ascontiguousarray(np.sin(rotary_pos_emb).T).astype(bf)
    # rotate_half as a matrix: rot(t)[2i] = -t[2i+1], rot(t)[2i+1] = t[2i]
    P = np.zeros((DH, DH), np.float32)
    for i in range(DH // 2):
        P[2 * i, 2 * i + 1] = -1.0
        P[2 * i + 1, 2 * i] = 1.0
    pm = np.ascontiguousarray(P.T).astype(bf)       # lhsT for rot matmul
    ident = np.eye(128, dtype=bf)
    onesm = np.ones((128, 128), dtype=bf)
    # mtri = M.T with M[jj, cc] = -1e30 where cc < jj (strict lower tri)
    M = np.where(np.arange(128)[None, :] < np.arange(128)[:, None],
                 np.float32(-1e30), np.float32(0.0))
    mtri = np.ascontiguousarray(M.T).astype(bf)

    Ws = (w_qkv * w_rms[None, :]).astype(np.float32)  # fold RMSNorm weight
    per_core = []
    for g in range(2):
        rq = Ws[g * 1024:(g + 1) * 1024]              # q rows, heads 8g..
        rk = Ws[D + g * 1024:D + (g + 1) * 1024]      # k rows
        rv = Ws[2 * D + g * 1024:2 * D + (g + 1) * 1024]
        wqk_g = np.concatenate([rq, rk], 0).T.astype(bf)   # [D, 2048]
        wv_g = rv.T.astype(bf)                             # [D, 1024]
        wout_g = w_out[:, g * 1024:(g + 1) * 1024].T.astype(bf)  # [EV, D]
        # repack so device DMAs are contiguous per partition:
        # wqk [D=(dt p), e] -> [p, et, dt, 128]
        wqk_pk = np.ascontiguousarray(
            wqk_g.reshape(16, 128, 16, 128).transpose(1, 2, 0, 3))
        # wv [D=(dt p), e] -> [p, ech, dt, 512]
        wv_pk = np.ascontiguousarray(
            wv_g.reshape(16, 128, 2, 512).transpose(1, 2, 0, 3))
        # wout [EV=(et p), d] -> [p, et, d]
        wout_pk = np.ascontiguousarray(
            wout_g.reshape(8, 128, 2048).transpose(1, 0, 2))
        per_core.append(dict(wqk=wqk_pk, wv=wv_pk, wout=wout_pk,
                             cos_t=cos_t, sin_t=sin_t, pm=pm, ident=ident,
                             onesm=onesm, mtri=mtri))
    return per_core


def kernel(x, rotary_pos_emb, w_rms, w_qkv, w_out, _run=None):
    x = np.asarray(x, np.float32)
    rotary_pos_emb = np.asarray(rotary_pos_emb, np.float32)
    w_rms = np.asarray(w_rms, np.float32)
    w_qkv = np.asarray(w_qkv, np.float32)
    w_out = np.asarray(w_out, np.float32)

    nc = build_nc()
    groups = _host_prep(rotary_pos_emb, w_rms, w_qkv, w_out)
    in_maps = []
    for b in range(B):
        for g in range(2):
            m = dict(groups[g])
            m["x"] = np.ascontiguousarray(x[b])
            in_maps.append(m)
    if _run is None:
        res = run_bass_kernel_spmd(nc, in_maps, core_ids=list(range(8)))
        results = res.results
    else:
        results = _run(nc, in_maps)

    y = np.empty((B, N, D), np.float32)
    for b in range(B):
        y[b] = results[2 * b]["y"] + results[2 * b + 1]["y"]
    return y
